# revision 10
# baseline (speedup 1.0000x reference)
"""Cosine-similarity loss on Trainium2 — 8-core SPMD Bass/Tile kernel (v5).

Math (per token, logits row l of length V, target t):
    probs = softmax(l);  cos = probs[t] / ||probs||_2
  The softmax normalizer cancels in the ratio:
    cos = exp(l_t) / sqrt(sum_i exp(2*l_i))
  loss = 1 - sum(cos * mask) / (sum(mask) + 1e-8),  mask = (t != 0)

Two-path vocab-sum (v4 trace: ACT busy 47us, DVE 47us, PE warm at
215ns/MM; tail chain 5.1us; Tile pre/postamble ~16.5us fixed):
  * ACT share (VA cols, token-major): native Exp, 1 elem/cycle/lane
    @1.2GHz, free fp32 accumulation (accum_out).  ~149 G elem/s.
  * PE share (VP rows, vocab-major staged transposed on host): one
    2x-mode DVE tensor_scalar makes int16(l*A16+B16) whose bit pattern
    IS exp(2l) in bf16 (~243 G elem/s); TensorEngine reduces along
    partitions (=vocab) via ones[128,1] matmuls accumulating into one
    PSUM row [1, 512tok] (215ns per 512-col MM, warm).
  * fp8e4m3 staging, 16.4 MB/core -> ~44-50us DMA; chunks ~1.2-1.5MB
    interleaved by engine consumption deadline; final PE chunk small to
    shrink the trailing-latency after the last DMA.
  * Tail (v5 compression): PSUM row -> bf16 SBUF -> 4 tiny bf16
    transpose matmuls -> s2 add; rsqrt = fast-inverse-sqrt bit trick
    WITHOUT the Newton step (loss err ~6e-5, tol 2e-2); numerator
    pre-masked (em = exp(l_t)*mask) and mask-sum reduced early, both
    off the critical path.
  * Numerator: 4 per-column indirect-DMA gathers (512 fp32) from the
    full-precision staged copy, issued early so SWDGE descriptor
    generation hides under the stream.
  * Mask derived on-device from gidx (iota token index * V == gidx <=> pad).

Sharding: tokens (B*S = 4096) split evenly across 8 NeuronCores, 512/core
(4 tiles of 128 partitions, token j at partition j%128, tile j//128).
Each core returns per-partition partials of cos*mask and mask; the host
adds 8x128 partials and finishes the division.
"""

import numpy as np
import ml_dtypes

import concourse.bacc as bacc
import concourse.bass as bass
import concourse.mybir as mybir
import concourse.tile as tile
from concourse.bass_utils import run_bass_kernel_spmd

B, S, V = 2, 2048, 32000
N_CORES = 8
NTOK = B * S                      # 4096
TOK_PER_CORE = NTOK // N_CORES    # 512
P = 128
TILES = TOK_PER_CORE // P         # 4 token tiles per core
EPS_MEAN = 1e-8

# vocab split between the two paths
VA = 12800                        # ACT share (token-major)
NA = 2                            # ACT chunks per tile row
CA = VA // NA                     # cols per ACT instruction
VP = V - VA                       # 19200 PE share (vocab-major)
NP = VP // P                      # 150 vocab tiles of 128
PE_CHUNKS = [6] + [18] * 8        # vocab tiles per chunk; first small so
                                  # the DVE stream starts early
assert sum(PE_CHUNKS) == NP

# Schraudolph constants for exp(2*l) in the int16/bf16 domain:
#   bits16 = round((2*l) * (2^23/ln2)/2^16 + (127*2^23 - C)/2^16)
SCHRAUD_C = 366393.0
A16 = 2.0 * float(1 << 23) / float(np.log(2.0)) / 65536.0
B16 = (127.0 * float(1 << 23) - SCHRAUD_C) / 65536.0 - 4.04  # -4.04: bias trim


def build_program():
    """Build + compile the per-core Bass program (identical on all cores)."""
    # NOTE: no num_devices — per-core programs are fully independent (the host
    # combines partials); num_devices>1 makes Tile emit a cross-device exit
    # barrier that crashes under the axon PJRT shim.
    nc = bacc.Bacc("TRN2", target_bir_lowering=False, debug=False)
    f32 = mybir.dt.float32
    i32 = mybir.dt.int32
    i16 = mybir.dt.int16
    bf16 = mybir.dt.bfloat16
    fp8 = mybir.dt.float8e4
    AF = mybir.ActivationFunctionType
    ALU = mybir.AluOpType
    AX = mybir.AxisListType

    l8a = nc.dram_tensor("l8a", [TOK_PER_CORE, VA], fp8, kind="ExternalInput").ap()
    l8p = nc.dram_tensor("l8p", [P, NP * TOK_PER_CORE], fp8, kind="ExternalInput").ap()
    lg = nc.dram_tensor("lg", [TOK_PER_CORE, V], f32, kind="ExternalInput").ap()
    gidx = nc.dram_tensor("gidx", [P, TILES], i32, kind="ExternalInput").ap()
    out = nc.dram_tensor("out", [P, 2], f32, kind="ExternalOutput").ap()

    # Element-gather view for the indirect DMA: [tok*v, 1] (DMA APs must be 2-D)
    lg_flat = lg.rearrange("a b -> (a b)").rearrange("(a b) -> a b", b=1)

    with tile.TileContext(nc) as tc:
        with (
            tc.tile_pool(name="adata", bufs=3) as adata,
            tc.tile_pool(name="pdata", bufs=3) as pdata,
            tc.tile_pool(name="ywork", bufs=2) as ywork,
            tc.tile_pool(name="small", bufs=1) as small,
            tc.tile_pool(name="psacc", bufs=1, space="PSUM") as psacc,
            tc.tile_pool(name="pstr", bufs=1, space="PSUM") as pstr,
        ):
            s2a = small.tile([P, TILES * NA], f32)
            res = small.tile([P, 2], f32)

            # PSUM accumulator row: per-token sum of exp(2l) over the PE share
            ps_row = psacc.tile([1, TOK_PER_CORE], f32)

            # stationary ones for the PE vocab reduction (bf16 for 1-pass MMs)
            ones_bf = small.tile([P, 1], bf16)
            nc.any.memset(ones_bf[:], 1.0)
            ones_b1 = small.tile([1, 1], bf16)
            nc.any.memset(ones_b1[:], 1.0)

            # --- all bulk DMAs issued upfront, self-clocked by the tile-pool
            # slot semaphores.  The two streams ride SEPARATE HWDGE rings
            # (ACT chunks on qActDynamicHW via nc.scalar, PE chunks + gidx +
            # out on qSPDynamicHW via nc.sync) so a slot-wait at one ring's
            # FIFO head can never starve the other engine's data (the v5
            # single-ring regression: 15us scalar stall).
            a_tiles = {}
            p_tiles = {}

            gidx_sb = small.tile([P, TILES], i32)
            nc.sync.dma_start(out=gidx_sb[:], in_=gidx)

            for i in range(TILES * NA):
                t, c = divmod(i, NA)
                ach = adata.tile([P, CA], fp8, tag="achunk")
                nc.scalar.dma_start(
                    out=ach[:],
                    in_=l8a[t * P : (t + 1) * P, c * CA : (c + 1) * CA],
                )
                a_tiles[i] = ach

            for j in range(len(PE_CHUNKS)):
                ntile = PE_CHUNKS[j]
                col0 = sum(PE_CHUNKS[:j]) * TOK_PER_CORE
                pch = pdata.tile([P, 18 * TOK_PER_CORE], fp8, tag="pchunk")
                nc.sync.dma_start(
                    out=pch[:, : ntile * TOK_PER_CORE],
                    in_=l8p[:, col0 : col0 + ntile * TOK_PER_CORE],
                )
                p_tiles[j] = pch

            # gathers next: SWDGE descriptor generation (~1.1us/col on the
            # gpsimd queue) hides under the stream.
            lt = small.tile([P, TILES], f32)
            for t in range(TILES):
                nc.gpsimd.indirect_dma_start(
                    out=lt[:, t : t + 1],
                    out_offset=None,
                    in_=lg_flat,
                    in_offset=bass.IndirectOffsetOnAxis(
                        ap=gidx_sb[:, t : t + 1], axis=0
                    ),
                )

            # mask inputs (device-derived): token base index via iota
            tokidx = small.tile([P, TILES], i32)
            nc.gpsimd.iota(
                out=tokidx[:], pattern=[[P, TILES]], base=0, channel_multiplier=1
            )
            gbase = small.tile([P, TILES], i32)
            nc.vector.tensor_scalar(
                out=gbase[:], in0=tokidx[:], scalar1=float(V), scalar2=None,
                op0=ALU.mult,
            )
            mask_sb = small.tile([P, TILES], f32)
            nc.vector.tensor_tensor(
                out=mask_sb[:], in0=gidx_sb[:], in1=gbase[:], op=ALU.not_equal
            )
            # mask-sum is independent of everything else: do it now
            nc.vector.tensor_reduce(
                out=res[:, 1:2], in_=mask_sb[:], axis=AX.X, op=ALU.add
            )

            # --- ACT share: one Exp+accum per token-tile row.  In-place fp8
            # output is clamped garbage nothing reads; the accumulated fp32
            # row sums are the real output.
            for i in range(TILES * NA):
                ach = a_tiles[i]
                nc.scalar.activation(
                    out=ach[:], in_=ach[:], func=AF.Exp, scale=2.0,
                    accum_out=s2a[:, i : i + 1],
                )

            # --- PE share: Schraudolph bits on DVE, vocab-sum on PE.
            mm_total = NP
            mm_done = 0
            for j in range(len(PE_CHUNKS)):
                ntile = PE_CHUNKS[j]
                w = ntile * TOK_PER_CORE
                pch = p_tiles[j]
                y16 = ywork.tile([P, 18 * TOK_PER_CORE], i16, tag="y16")
                nc.vector.tensor_scalar(
                    out=y16[:, :w], in0=pch[:, :w], scalar1=float(A16),
                    scalar2=float(B16), op0=ALU.mult, op1=ALU.add,
                )
                yb = y16[:].bitcast(bf16)
                for k in range(ntile):
                    nc.tensor.matmul(
                        ps_row[:1, :],
                        ones_bf[:],
                        yb[:, k * TOK_PER_CORE : (k + 1) * TOK_PER_CORE],
                        start=(mm_done == 0),
                        stop=(mm_done == mm_total - 1),
                    )
                    mm_done += 1

            # --- PE-share drain: PSUM row -> bf16 SBUF -> token-major
            # [128, TILES] via 4 tiny 1-pass bf16 transpose matmuls.
            s2row = small.tile([1, TOK_PER_CORE], bf16)
            nc.vector.tensor_copy(s2row[:], ps_row[:1, :])
            ps_t = pstr.tile([P, TILES], f32)
            for t in range(TILES):
                nc.tensor.matmul(
                    ps_t[:, t : t + 1],
                    s2row[:1, t * P : (t + 1) * P],
                    ones_b1[:1, :],
                    start=True, stop=True,
                )
            s2p = small.tile([P, TILES], f32)
            nc.vector.tensor_copy(s2p[:], ps_t[:])

            # --- numerator exp, pre-masked (off the critical path) ---
            exp_lt = small.tile([P, TILES], f32)
            nc.scalar.activation(out=exp_lt[:], in_=lt[:], func=AF.Exp)
            em = small.tile([P, TILES], f32)
            nc.vector.tensor_mul(em[:], exp_lt[:], mask_sb[:])

            s2_1 = small.tile([P, TILES], f32)
            nc.vector.tensor_reduce(
                out=s2_1[:],
                in_=s2a[:].rearrange("p (t c) -> p t c", c=NA),
                axis=AX.X, op=ALU.add,
            )
            s2 = small.tile([P, TILES], f32)
            nc.vector.tensor_add(s2[:], s2_1[:], s2p[:])

            # rs ~= 1/sqrt(s2): fast-inverse-sqrt bit trick, no Newton step
            # (y0 rel err in [-3.4%, +1.2%] -> ~6e-5 on the loss; tol 2e-2).
            sh = small.tile([P, TILES], i32)
            nc.vector.tensor_scalar(
                out=sh[:], in0=s2[:].bitcast(i32), scalar1=1, scalar2=None,
                op0=ALU.arith_shift_right,
            )
            y0i = small.tile([P, TILES], i32)
            nc.vector.tensor_scalar(
                out=y0i[:], in0=sh[:], scalar1=-1.0, scalar2=float(0x5F3759DF),
                op0=ALU.mult, op1=ALU.add,
            )
            cosm = small.tile([P, TILES], f32)
            nc.vector.tensor_mul(cosm[:], em[:], y0i[:].bitcast(f32))

            nc.vector.tensor_reduce(
                out=res[:, 0:1], in_=cosm[:], axis=AX.X, op=ALU.add
            )
            nc.sync.dma_start(out=out, in_=res[:])

    nc.compile()
    return nc


_NC_CACHE = {}


def _get_nc():
    if "nc" not in _NC_CACHE:
        _NC_CACHE["nc"] = build_program()
    return _NC_CACHE["nc"]


def make_in_maps(logits, targets):
    """Shard full inputs into per-core input maps (host-side prep only)."""
    logits = np.asarray(logits)
    targets = np.asarray(targets)
    assert logits.shape == (B, S, V), logits.shape
    lf = np.ascontiguousarray(logits.reshape(NTOK, V).astype(np.float32, copy=False))
    l8f = lf.astype(ml_dtypes.float8_e4m3fn)
    tf = targets.reshape(NTOK).astype(np.int64)

    # token j of a core sits at (partition p = j % P, tile t = j // P)
    local_tok = (np.arange(TILES)[None, :] * P + np.arange(P)[:, None]).astype(np.int64)

    in_maps = []
    for k in range(N_CORES):
        sl = slice(k * TOK_PER_CORE, (k + 1) * TOK_PER_CORE)
        tk = tf[sl].reshape(TILES, P).T          # [P, TILES]
        gidx = (local_tok * V + tk).astype(np.int32)
        blk8 = l8f[sl]                            # [512, V]
        # PE share staged vocab-major: l8p[p, j*512+t] = l[t, VA + j*128 + p]
        l8p = np.ascontiguousarray(
            blk8[:, VA:].reshape(TOK_PER_CORE, NP, P).transpose(2, 1, 0)
            .reshape(P, NP * TOK_PER_CORE)
        )
        in_maps.append(
            {
                "l8a": np.ascontiguousarray(blk8[:, :VA]),
                "l8p": l8p,
                "lg": lf[sl],
                "gidx": np.ascontiguousarray(gidx),
            }
        )
    return in_maps


def reduce_outputs(per_core_outs):
    """Combine per-core [128, 2] partials into the final scalar loss."""
    s = 0.0
    c = 0.0
    for o in per_core_outs:
        s += float(o[:, 0].astype(np.float64).sum())
        c += float(o[:, 1].astype(np.float64).sum())
    return np.asarray(np.float32(1.0 - s / (c + EPS_MEAN)))


def run_on_device(in_maps, **kwargs):
    nc = _get_nc()
    return run_bass_kernel_spmd(nc, in_maps, core_ids=list(range(N_CORES)), **kwargs)


def kernel(logits, targets):
    in_maps = make_in_maps(logits, targets)
    res = run_on_device(in_maps)
    return reduce_outputs([r["out"] for r in res.results])
